# revision 10
# baseline (speedup 1.0000x reference)
"""Trainium2 Bass kernel for nn_DenseRMoK (RevIN + softmax-gated mixture of
Taylor/Wavelet KAN experts), data-parallel over the B*N row dimension on 8
NeuronCores.

Self-contained: hardcodes shapes/sharding, builds + runs the Bass program via
run_bass_kernel_spmd, gathers to the full [B, P, N] output.

Math (per flattened row r=(b,n), xf = RevIN-normalized x[b,:,n], L=512):
  score  = softmax(xf @ gate_w.T + gate_b)                         [E=4]
  taylor_e = sum_l c0[p,l] + xf @ c1.T + xf^2 @ c2.T + bias        [P=96]
  wave_e   = MH*(xf^2-1)*exp(-xf^2/2) @ ww.T (scaled by BN gamma)  [P=96]
  pred   = sum_e eo_e * score_e ; out = denorm(pred)

The wave experts' per-(p,l) scale/translation are identity in this model
(scale=1, trans=0), which makes the wavelet basis rank-1 in (row, l); the
kernel checks this at runtime and falls back to an exact numpy path if not.

Device layout: everything lives feature-major ([l or feature, row]); RevIN
stats come from a ones-vector matmul, per-row norm coefficients are broadcast
across partitions with a K=1 matmul, expert outputs are PE-transposed to
row-major for the score-weighted mixture, and softmax normalization + RevIN
denorm are folded into one per-row scale/bias pair.
"""

import math
import sys

import numpy as np

if "/opt/trn_rl_repo" not in sys.path:
    sys.path.insert(0, "/opt/trn_rl_repo")

# Problem dims (fixed by the model)
B, L, N, P, E = 32, 512, 64, 96, 4
EPS = 1e-5
BN_EPS = 1e-5
MH = 2.0 / (math.sqrt(3.0) * math.pi**0.25)

NCORES = 8
BPC = B // NCORES  # batches per core
R = BPC * N        # 256 rows per core
PD = 128           # SBUF partitions
NCH = L // PD      # 4 contraction chunks

WCOLS = NCH * P    # columns per [L, P] weight in chunk layout
WTOT = 6 * WCOLS + NCH * E  # packed weight tensor columns
VECN = 3 * R + 1 + PD       # rw | rb | rvi | eps | ones-row

_NC_CACHE = {}


def _build_nc(debug=False):
    """Build the single-core Bass/Tile program (SPMD across 8 cores)."""
    import concourse.tile as tile
    from concourse import bacc, mybir
    from concourse._compat import get_trn_type

    f32 = mybir.dt.float32
    bf16 = mybir.dt.bfloat16
    AF = mybir.ActivationFunctionType
    OP = mybir.AluOpType

    nc = bacc.Bacc(get_trn_type() or "TRN2", target_bir_lowering=False, debug=debug)

    x_d = nc.dram_tensor("x", [PD, NCH * R], f32, kind="ExternalInput")
    w_d = nc.dram_tensor("w", [PD, WTOT], bf16, kind="ExternalInput")
    id_d = nc.dram_tensor("ident", [PD, PD + 2], f32, kind="ExternalInput")
    cst_d = nc.dram_tensor("cst", [P, 5], f32, kind="ExternalInput")
    vec_d = nc.dram_tensor("vec", [1, VECN], f32, kind="ExternalInput")
    out_d = nc.dram_tensor("out", [P, R], f32, kind="ExternalOutput")

    with tile.TileContext(nc) as tc:
        with (
            tc.tile_pool(name="const", bufs=1) as cp,
            tc.tile_pool(name="big", bufs=1) as bp,
            tc.tile_pool(name="small", bufs=2) as sp,
            tc.tile_pool(name="pacc", bufs=2, space="PSUM") as pacc,
            tc.tile_pool(name="pmisc", bufs=2, space="PSUM") as pmisc,
            tc.tile_pool(name="ptr", bufs=3, space="PSUM") as ptr,
        ):
            dma = nc.sync.dma_start

            idsb = cp.tile([PD, PD + 2], f32)
            dma(out=idsb, in_=id_d[:])
            ident = idsb[:, 0:PD]
            onescol = idsb[:, PD : PD + 1]
            zcol = idsb[:, PD + 1 : PD + 2]

            cst = cp.tile([P, 5], f32)
            dma(out=cst, in_=cst_d[:])
            gb = cst[0:E, 4:5]

            vec = cp.tile([1, VECN], f32)
            dma(out=vec, in_=vec_d[:])
            rw = vec[:, 0:R]
            rb = vec[:, R : 2 * R]
            rvi = vec[:, 2 * R : 3 * R]
            epsT = vec[:, 3 * R : 3 * R + 1]
            onesr = vec[:, 3 * R + 1 : 3 * R + 1 + PD]

            wsb = cp.tile([PD, WTOT], bf16)
            dma(out=wsb, in_=w_d[:])
            c10 = wsb[:, 0 * WCOLS : 1 * WCOLS]
            c20 = wsb[:, 1 * WCOLS : 2 * WCOLS]
            c11 = wsb[:, 2 * WCOLS : 3 * WCOLS]
            c21 = wsb[:, 3 * WCOLS : 4 * WCOLS]
            ww0 = wsb[:, 4 * WCOLS : 5 * WCOLS]
            ww1 = wsb[:, 5 * WCOLS : 6 * WCOLS]
            wg = wsb[:, 6 * WCOLS : 6 * WCOLS + NCH * E]

            xs = bp.tile([PD, NCH * R], f32)
            dma(out=xs, in_=x_d[:])

            # --- RevIN stats: per-row sums of x and x^2 via ones-matmul.
            # Squaring + chunk pre-sums run on gpsimd (otherwise idle).
            sq = bp.tile([PD, NCH * R], f32)
            nc.gpsimd.tensor_mul(sq, xs, xs)
            s2 = bp.tile([PD, 2 * R], f32)
            nc.gpsimd.tensor_add(s2[:, 0:R], xs[:, 0:R], xs[:, R : 2 * R])
            nc.gpsimd.tensor_add(s2[:, 0:R], s2[:, 0:R], xs[:, 2 * R : 3 * R])
            nc.gpsimd.tensor_add(s2[:, 0:R], s2[:, 0:R], xs[:, 3 * R : 4 * R])
            nc.gpsimd.tensor_add(s2[:, R:], sq[:, 0:R], sq[:, R : 2 * R])
            nc.gpsimd.tensor_add(s2[:, R:], s2[:, R:], sq[:, 2 * R : 3 * R])
            nc.gpsimd.tensor_add(s2[:, R:], s2[:, R:], sq[:, 3 * R : 4 * R])

            pst = pmisc.tile([1, 2 * R], f32, tag="m")
            nc.tensor.matmul(pst, onescol, s2, start=True, stop=True)
            stat = sp.tile([1, 2 * R], f32, tag="stat")
            nc.scalar.activation(stat, pst, AF.Copy, scale=1.0 / L)
            mean = stat[:, 0:R]
            ex2 = stat[:, R:]
            var = sp.tile([1, R], f32, tag="var")
            nc.vector.tensor_mul(var, mean, mean)
            nc.vector.tensor_sub(var, ex2, var)
            stdr = sp.tile([1, R], f32, tag="stdr")
            nc.scalar.activation(stdr, var, AF.Sqrt, bias=epsT)
            istd = sp.tile([1, R], f32, tag="istd")
            nc.vector.reciprocal(istd, stdr)

            # norm coeffs: xn = a*x + c ; denorm: out = da2*pred_raw + dc
            acr = sp.tile([1, 2 * R], f32, tag="acr")
            a_r = acr[:, 0:R]
            c_r = acr[:, R:]
            nc.vector.tensor_mul(a_r, istd, rw)
            nc.vector.tensor_mul(c_r, mean, a_r)
            nc.vector.tensor_sub(c_r, rb, c_r)
            dar = sp.tile([1, R], f32, tag="dar")
            dcr = sp.tile([1, R], f32, tag="dcr")
            nc.vector.tensor_mul(dar, stdr, rvi)
            nc.vector.tensor_mul(dcr, rb, dar)
            nc.vector.tensor_sub(dcr, mean, dcr)

            # broadcast a|c across partitions via K=1 matmul
            pac = pmisc.tile([PD, 2 * R], f32, tag="m")
            nc.tensor.matmul(pac, onesr, acr, start=True, stop=True)
            AC = bp.tile([PD, 2 * R], f32)
            nc.scalar.copy(AC, pac)

            xn = bp.tile([PD, NCH * R], f32)
            for c in range(NCH):
                cs = slice(c * R, (c + 1) * R)
                nc.vector.tensor_mul(xn[:, cs], xs[:, cs], AC[:, 0:R])
                nc.vector.tensor_add(xn[:, cs], xn[:, cs], AC[:, R:])

            x2 = bp.tile([PD, NCH * R], f32)
            nc.vector.tensor_mul(x2, xn, xn)
            eT = bp.tile([PD, NCH * R], f32)
            nc.scalar.activation(eT, x2, AF.Exp, bias=zcol, scale=-0.5)
            # bf16 matmul operands: psi comes out bf16 directly
            psib = bp.tile([PD, NCH * R], bf16)
            nc.vector.scalar_tensor_tensor(psib, x2, -1.0, eT, op0=OP.add, op1=OP.mult)
            xnb = bp.tile([PD, NCH * R], bf16)
            nc.scalar.copy(xnb, xn)
            x2b = bp.tile([PD, NCH * R], bf16)
            nc.gpsimd.tensor_copy(x2b, x2)

            # --- gate: logits -> exp (softmax normalization folded into denorm) ---
            pg = pmisc.tile([E, R], f32, tag="m")
            for c in range(NCH):
                nc.tensor.matmul(
                    pg,
                    wg[:, c * E : (c + 1) * E],
                    xnb[:, c * R : (c + 1) * R],
                    start=(c == 0),
                    stop=(c == NCH - 1),
                )
            expg = sp.tile([E, R], f32, tag="expg")
            nc.scalar.activation(expg, pg, AF.Exp, bias=gb)
            prs = pmisc.tile([1, R], f32, tag="m")
            nc.tensor.matmul(prs, onescol[0:E, 0:1], expg, start=True, stop=True)
            rinv = sp.tile([1, R], f32, tag="rinv")
            nc.vector.reciprocal(rinv, prs)
            nc.vector.tensor_mul(dar, dar, rinv)

            # --- experts: accumulate over l-chunks in PSUM (bf16 inputs) ---
            eo_sb = []
            plans = [
                ([(c10, xnb), (c20, x2b)], cst[:, 0:1]),
                ([(c11, xnb), (c21, x2b)], cst[:, 1:2]),
                ([(ww0, psib)], cst[:, 2:3]),
                ([(ww1, psib)], cst[:, 3:4]),
            ]
            for ei, (mms, bcol) in enumerate(plans):
                pe_ = pacc.tile([P, R], f32, tag="acc")
                nmm = len(mms) * NCH
                i = 0
                for w, rhs in mms:
                    for c in range(NCH):
                        nc.tensor.matmul(
                            pe_,
                            w[:, c * P : (c + 1) * P],
                            rhs[:, c * R : (c + 1) * R],
                            start=(i == 0),
                            stop=(i == nmm - 1),
                        )
                        i += 1
                eo = sp.tile([P, R], f32, tag=f"eo{ei}")
                nc.vector.tensor_scalar_add(eo, pe_, bcol)
                eo_sb.append(eo)

            # --- transpose to row-major, mix experts, denorm, transpose back ---
            outp = bp.tile([P, R], f32)
            for j in range(R // PD):
                js = slice(j * PD, (j + 1) * PD)
                ptg = ptr.tile([PD, E], f32, tag="tr")
                nc.tensor.transpose(ptg, expg[:, js], ident[0:E, 0:E])
                sg = sp.tile([PD, E], f32, tag="sg")
                nc.scalar.copy(sg, ptg)
                ptd = ptr.tile([PD, 1], f32, tag="tr")
                nc.tensor.transpose(ptd, dar[:, js], ident[0:1, 0:1])
                ptd2 = ptr.tile([PD, 1], f32, tag="tr")
                nc.tensor.transpose(ptd2, dcr[:, js], ident[0:1, 0:1])
                dcol = sp.tile([PD, 2], f32, tag="dcol")
                nc.scalar.copy(dcol[:, 0:1], ptd)
                nc.scalar.copy(dcol[:, 1:2], ptd2)
                sgp = sp.tile([PD, E], f32, tag="sgp")
                nc.vector.tensor_scalar_mul(sgp, sg, dcol[:, 0:1])
                pred = sp.tile([PD, P], f32, tag="pred")
                for e in range(E):
                    pteo = ptr.tile([PD, P], f32, tag="tr")
                    nc.tensor.transpose(pteo, eo_sb[e][:, js], ident[0:P, 0:P])
                    if e == 0:
                        nc.vector.tensor_scalar(
                            pred,
                            pteo,
                            sgp[:, 0:1],
                            dcol[:, 1:2],
                            op0=OP.mult,
                            op1=OP.add,
                        )
                    else:
                        nc.vector.scalar_tensor_tensor(
                            pred, pteo, sgp[:, e : e + 1], pred, op0=OP.mult, op1=OP.add
                        )
                pto = ptr.tile([P, PD], f32, tag="tr")
                nc.tensor.transpose(pto, pred, ident)
                nc.scalar.copy(outp[:, js], pto)

            dma(out=out_d[:], in_=outp)

    nc.compile()
    return nc


def _chunked(wT):
    """[L, M] -> [128, NCH*M] with column block c holding rows l=c*128.."""
    Lx, M = wT.shape
    return np.ascontiguousarray(
        wT.reshape(NCH, PD, M).transpose(1, 0, 2).reshape(PD, NCH * M)
    )


def _host_prep(inputs):
    import ml_dtypes

    f = np.float32
    bf = ml_dtypes.bfloat16
    g = {k: np.asarray(v, f) for k, v in inputs.items()}

    bn_scale = MH / math.sqrt(1.0 + BN_EPS)
    wparts = [
        _chunked(np.ascontiguousarray(g["t0_coeffs"][:, :, 1].T)),
        _chunked(np.ascontiguousarray(g["t0_coeffs"][:, :, 2].T)),
        _chunked(np.ascontiguousarray(g["t1_coeffs"][:, :, 1].T)),
        _chunked(np.ascontiguousarray(g["t1_coeffs"][:, :, 2].T)),
        _chunked(
            np.ascontiguousarray((g["w0_ww"] * g["w0_gamma"][:, None] * bn_scale).T)
        ),
        _chunked(
            np.ascontiguousarray((g["w1_ww"] * g["w1_gamma"][:, None] * bn_scale).T)
        ),
        _chunked(np.ascontiguousarray(g["gate_w"].T)),
    ]
    w_h = np.concatenate(wparts, axis=1).astype(bf)
    assert w_h.shape == (PD, WTOT)

    id_h = np.zeros((PD, PD + 2), f)
    id_h[:, 0:PD] = np.eye(PD, dtype=f)
    id_h[:, PD] = 1.0

    cst_h = np.zeros((P, 5), f)
    cst_h[:, 0] = (
        g["t0_coeffs"][:, :, 0].sum(axis=1, dtype=np.float64) + g["t0_bias"][0]
    ).astype(f)
    cst_h[:, 1] = (
        g["t1_coeffs"][:, :, 0].sum(axis=1, dtype=np.float64) + g["t1_bias"][0]
    ).astype(f)
    cst_h[:, 2] = g["w0_beta"]
    cst_h[:, 3] = g["w1_beta"]
    cst_h[0:E, 4] = g["gate_b"]

    vec_h = np.zeros((1, VECN), f)
    vec_h[0, 0:R] = np.tile(g["rev_w"], BPC)
    vec_h[0, R : 2 * R] = np.tile(g["rev_b"], BPC)
    vec_h[0, 2 * R : 3 * R] = np.tile(1.0 / (g["rev_w"] + f(EPS)), BPC)
    vec_h[0, 3 * R] = EPS
    vec_h[0, 3 * R + 1 :] = 1.0

    common = {"w": w_h, "ident": id_h, "cst": cst_h, "vec": vec_h}

    x = g["x"]
    xcores = []
    for i in range(NCORES):
        xc = x[i * BPC : (i + 1) * BPC]  # [BPC, L, N]
        xcores.append(
            np.ascontiguousarray(
                xc.reshape(BPC, NCH, PD, N).transpose(2, 1, 0, 3).reshape(PD, NCH * R)
            )
        )
    return common, xcores


def _fast_ok(inputs):
    try:
        return (
            np.all(np.asarray(inputs["w0_scale"]) == 1.0)
            and np.all(np.asarray(inputs["w1_scale"]) == 1.0)
            and np.all(np.asarray(inputs["w0_trans"]) == 0.0)
            and np.all(np.asarray(inputs["w1_trans"]) == 0.0)
        )
    except Exception:
        return False


def _numpy_ref(inputs):
    """Exact general fallback (host numpy), mirrors the reference module."""
    g = {k: np.asarray(v, np.float32) for k, v in inputs.items()}
    x = g["x"]
    mean = x.mean(axis=1, keepdims=True)
    stdev = np.sqrt(x.var(axis=1, keepdims=True) + np.float32(EPS))
    xn = (x - mean) / stdev * g["rev_w"] + g["rev_b"]
    xf = xn.transpose(0, 2, 1).reshape(B * N, L)
    logits = xf @ g["gate_w"].T + g["gate_b"]
    logits -= logits.max(axis=-1, keepdims=True)
    elg = np.exp(logits)
    score = elg / elg.sum(axis=-1, keepdims=True)

    def taylor(c, b):
        y = np.full((B * N, P), c[:, :, 0].sum(axis=1), np.float32)
        y += xf @ c[:, :, 1].T + (xf * xf) @ c[:, :, 2].T
        return y + b

    def wave(s, t, w, gam, bet):
        y = np.empty((B * N, P), np.float32)
        for i0 in range(0, B * N, 128):
            xs = (xf[i0 : i0 + 128, None, :] - t[None]) / s[None]
            x2 = xs * xs
            psi = np.float32(MH) * (x2 - 1.0) * np.exp(-0.5 * x2)
            y[i0 : i0 + 128] = np.einsum("bpl,pl->bp", psi, w)
        return (y / np.sqrt(np.float32(1.0 + BN_EPS))) * gam + bet

    eo = np.stack(
        [
            taylor(g["t0_coeffs"], g["t0_bias"][0]),
            taylor(g["t1_coeffs"], g["t1_bias"][0]),
            wave(g["w0_scale"], g["w0_trans"], g["w0_ww"], g["w0_gamma"], g["w0_beta"]),
            wave(g["w1_scale"], g["w1_trans"], g["w1_ww"], g["w1_gamma"], g["w1_beta"]),
        ],
        axis=-1,
    )
    pred = np.einsum("bpE,bE->bp", eo, score)
    pred = pred.reshape(B, N, P).transpose(0, 2, 1)
    out = ((pred - g["rev_b"]) / (g["rev_w"] + np.float32(EPS))) * stdev + mean
    return out.astype(np.float32)


def run(inputs, trace=False):
    """Run the Bass kernel on 8 cores. Returns (out [B,P,N], exec_time_ns|None)."""
    from concourse.bass_utils import run_bass_kernel_spmd

    if "nc" not in _NC_CACHE:
        _NC_CACHE["nc"] = _build_nc()
    nc = _NC_CACHE["nc"]
    common, xcores = _host_prep(inputs)
    in_maps = [dict(common, x=xcores[i]) for i in range(NCORES)]
    try:
        res = run_bass_kernel_spmd(nc, in_maps, list(range(NCORES)), trace=trace)
    except ModuleNotFoundError:
        # NTFF profiling hooks unavailable in this image — run without trace.
        res = run_bass_kernel_spmd(nc, in_maps, list(range(NCORES)), trace=False)
    out = np.empty((B, P, N), np.float32)
    for i in range(NCORES):
        o = np.asarray(res.results[i]["out"]).reshape(P, BPC, N)
        out[i * BPC : (i + 1) * BPC] = o.transpose(1, 0, 2)
    return out, res.exec_time_ns


def kernel(**inputs):
    if not _fast_ok(inputs):
        return _numpy_ref(inputs)
    out, _ = run(inputs)
    return out


# revision 42
# speedup vs baseline: 2051.7308x; 2051.7308x over previous
"""Trainium2 Bass kernel for nn_DenseRMoK (RevIN + softmax-gated mixture of
Taylor/Wavelet KAN experts), data-parallel over the B*N row dimension on 8
NeuronCores.

Self-contained: hardcodes shapes/sharding, builds + runs the Bass program via
run_bass_kernel_spmd, gathers to the full [B, P, N] output.

Math (per flattened row r=(b,n), xf = RevIN-normalized x[b,:,n], L=512):
  score  = softmax(xf @ gate_w.T + gate_b)                         [E=4]
  taylor_e = sum_l c0[p,l] + xf @ c1.T + xf^2 @ c2.T + bias        [P=96]
  wave_e   = MH*(xf^2-1)*exp(-xf^2/2) @ ww.T (scaled by BN gamma)  [P=96]
  pred   = sum_e eo_e * score_e ; out = denorm(pred)

The wave experts' per-(p,l) scale/translation are identity in this model
(scale=1, trans=0), which makes the wavelet basis rank-1 in (row, l); the
kernel checks this at runtime and falls back to an exact numpy path if not.

Device layout: everything lives feature-major ([l or feature, row]); RevIN
stats come from a ones-vector matmul, per-row norm coefficients are broadcast
across partitions with a K=1 matmul, expert outputs are PE-transposed to
row-major for the score-weighted mixture, and softmax normalization + RevIN
denorm are folded into one per-row scale/bias pair.
"""

import math
import sys

import numpy as np

if "/opt/trn_rl_repo" not in sys.path:
    sys.path.insert(0, "/opt/trn_rl_repo")

# Problem dims (fixed by the model)
B, L, N, P, E = 32, 512, 64, 96, 4
EPS = 1e-5
BN_EPS = 1e-5
MH = 2.0 / (math.sqrt(3.0) * math.pi**0.25)

NCORES = 8
BPC = B // NCORES  # batches per core
R = BPC * N        # 256 rows per core
PD = 128           # SBUF partitions
NCH = L // PD      # 4 contraction chunks

WCOLS = NCH * P    # columns per [L, P] weight in chunk layout
WBASE = 6 * WCOLS + NCH * E  # packed weight columns (experts + gate)
WTOT = WBASE + 2 + PD  # + f32r ones col + 1/L col + f32r ones row (row 0)
VECN = 3 * R + 1 + PD       # rw | rb | rvi | eps | ones-row

_NC_CACHE = {}


def _build_nc(debug=False, loop_n=1):
    """Build the single-core Bass/Tile program (SPMD across 8 cores).

    loop_n > 1 wraps the whole body in a hardware For-loop — used only for
    timing (amortizes host dispatch overhead to expose per-iteration time).
    """
    from contextlib import nullcontext

    import concourse.tile as tile
    from concourse import bacc, mybir
    from concourse._compat import get_trn_type

    f32 = mybir.dt.float32
    f32r = mybir.dt.float32r
    AF = mybir.ActivationFunctionType
    OP = mybir.AluOpType

    nc = bacc.Bacc(get_trn_type() or "TRN2", target_bir_lowering=False, debug=debug)

    x_d = nc.dram_tensor("x", [PD, NCH * R], f32r, kind="ExternalInput")
    w_d = nc.dram_tensor("w", [PD, WTOT], f32r, kind="ExternalInput")
    id_d = nc.dram_tensor("ident", [PD, PD + 2], f32, kind="ExternalInput")
    cst_d = nc.dram_tensor("cst", [P, 5], f32, kind="ExternalInput")
    vec_d = nc.dram_tensor("vec", [1, VECN], f32, kind="ExternalInput")
    out_d = nc.dram_tensor("out", [P, R], f32, kind="ExternalOutput")

    with tile.TileContext(nc) as tc:
        with (
            tc.tile_pool(name="const", bufs=1) as cp,
            tc.tile_pool(name="big", bufs=1) as bp,
            tc.tile_pool(name="small", bufs=2) as sp,
            tc.tile_pool(name="pacc", bufs=2, space="PSUM") as pacc,
            tc.tile_pool(name="pmisc", bufs=3, space="PSUM") as pmisc,
            tc.tile_pool(name="ptr", bufs=3, space="PSUM") as ptr,
            tc.For_i(0, loop_n, 1) if loop_n > 1 else nullcontext(),
        ):
            dma = nc.sync.dma_start

            # x first on the HWDGE queue so nothing queues ahead of it
            xs = bp.tile([PD, NCH * R], f32r)
            sq = bp.tile([PD, NCH * R], f32r)
            for c in range(NCH):
                cs = slice(c * R, (c + 1) * R)
                dma(out=xs[:, cs], in_=x_d[:, cs])

            idsb = cp.tile([PD, PD + 2], f32)
            dma(out=idsb, in_=id_d[:])
            ident = idsb[:, 0:PD]
            onescol = idsb[:, PD : PD + 1]
            zcol = idsb[:, PD + 1 : PD + 2]

            cst = cp.tile([P, 5], f32)
            dma(out=cst, in_=cst_d[:])
            gb = cst[0:E, 4:5]

            vec = cp.tile([1, VECN], f32)
            dma(out=vec, in_=vec_d[:])
            rw = vec[:, 0:R]
            rb = vec[:, R : 2 * R]
            rvi = vec[:, 2 * R : 3 * R]
            epsT = vec[:, 3 * R : 3 * R + 1]
            onesr = vec[:, 3 * R + 1 : 3 * R + 1 + PD]

            # ACT table preloads: first ops on the engine, overlap the DMAs
            warm = cp.tile([1, 5], f32)
            nc.vector.memset(warm, 0.0)
            nc.scalar.activation(warm[:, 1:2], warm[:, 1:2], AF.Copy)
            for wi, wf in enumerate([AF.Sqrt, AF.Exp]):
                nc.scalar.activation(
                    warm[:, wi + 2 : wi + 3], warm[:, wi + 2 : wi + 3], wf,
                    bias=warm[:, 0:1],
                )

            # weights on the SWDGE queue, in parallel with x on HWDGE
            wsb = cp.tile([PD, WTOT], f32r)
            nc.gpsimd.dma_start(out=wsb, in_=w_d[:])
            c10 = wsb[:, 0 * WCOLS : 1 * WCOLS]
            c20 = wsb[:, 1 * WCOLS : 2 * WCOLS]
            c11 = wsb[:, 2 * WCOLS : 3 * WCOLS]
            c21 = wsb[:, 3 * WCOLS : 4 * WCOLS]
            ww0 = wsb[:, 4 * WCOLS : 5 * WCOLS]
            ww1 = wsb[:, 5 * WCOLS : 6 * WCOLS]
            wg = wsb[:, 6 * WCOLS : 6 * WCOLS + NCH * E]
            onescol_r = wsb[:, WBASE : WBASE + 1]
            invL_r = wsb[:, WBASE + 1 : WBASE + 2]
            onesr_r = wsb[0:1, WBASE + 2 : WBASE + 2 + PD]

            # per-chunk squares + accumulating stats matmuls pipeline with the
            # x DMA; 1/L is folded into the stats lhsT so PSUM holds mean|E[x^2]
            pstx = pmisc.tile([1, R], f32, tag="m")
            pstq = pmisc.tile([1, R], f32, tag="m")
            for c in range(NCH):
                cs = slice(c * R, (c + 1) * R)
                nc.vector.tensor_mul(sq[:, cs], xs[:, cs], xs[:, cs])
                nc.tensor.matmul(
                    pstx, invL_r, xs[:, cs],
                    start=(c == 0), stop=(c == NCH - 1),
                )
                nc.tensor.matmul(
                    pstq, invL_r, sq[:, cs],
                    start=(c == 0), stop=(c == NCH - 1),
                )
            ex2 = pstq[0:1, 0:R]
            mean = sp.tile([1, R], f32, tag="mean")
            nc.vector.tensor_copy(mean, pstx[0:1, 0:R])
            var = sp.tile([1, R], f32, tag="var")
            nc.vector.tensor_mul(var, mean, mean)
            nc.vector.tensor_sub(var, ex2, var)
            stdr = sp.tile([1, R], f32, tag="stdr")
            nc.scalar.activation(stdr, var, AF.Sqrt, bias=epsT)
            istd = sp.tile([1, R], f32, tag="istd")
            nc.vector.reciprocal(istd, stdr)

            # norm coeffs: xn = a*x + c ; denorm: out = da2*pred_raw + dc
            acr = sp.tile([1, 2 * R], f32r, tag="acr")
            a_r = acr[:, 0:R]
            c_r = acr[:, R:]
            nc.vector.tensor_mul(a_r, istd, rw)
            nc.vector.tensor_mul(c_r, mean, a_r)
            nc.vector.tensor_sub(c_r, rb, c_r)
            dar = sp.tile([1, R], f32, tag="dar")
            dcr = sp.tile([1, R], f32, tag="dcr")
            nc.vector.tensor_mul(dar, stdr, rvi)
            nc.vector.tensor_mul(dcr, rb, dar)
            nc.vector.tensor_sub(dcr, mean, dcr)

            # broadcast a|c across partitions via K=1 matmul
            pac = pmisc.tile([PD, 2 * R], f32, tag="m")
            nc.tensor.matmul(pac, onesr_r, acr, start=True, stop=True)
            AC = bp.tile([PD, 2 * R], f32)
            nc.scalar.copy(AC, pac)

            # xn/x2/psi live as fp32r (fast PE matmuls: 1 cyc/row at N>=256),
            # emitted per l-chunk so DVE/ACT/PE pipeline across chunks
            xn = bp.tile([PD, NCH * R], f32r)
            x2 = bp.tile([PD, NCH * R], f32r)
            eT = bp.tile([PD, NCH * R], f32)
            psib = bp.tile([PD, NCH * R], f32r)
            for c in range(NCH):
                cs = slice(c * R, (c + 1) * R)
                nc.vector.tensor_mul(xn[:, cs], xs[:, cs], AC[:, 0:R])
                nc.vector.tensor_add(xn[:, cs], xn[:, cs], AC[:, R:])
                nc.vector.tensor_mul(x2[:, cs], xn[:, cs], xn[:, cs])
                nc.scalar.activation(eT[:, cs], x2[:, cs], AF.Exp, bias=zcol, scale=-0.5)
                nc.vector.scalar_tensor_tensor(
                    psib[:, cs], x2[:, cs], -1.0, eT[:, cs], op0=OP.add, op1=OP.mult
                )
            xnb = xn
            x2b = x2

            # --- gate: logits -> exp (softmax normalization folded into denorm) ---
            pg = pmisc.tile([E, R], f32, tag="m")
            for c in range(NCH):
                nc.tensor.matmul(
                    pg,
                    wg[:, c * E : (c + 1) * E],
                    xnb[:, c * R : (c + 1) * R],
                    start=(c == 0),
                    stop=(c == NCH - 1),
                )
            expg = sp.tile([E, R], f32, tag="expg")
            nc.scalar.activation(expg, pg, AF.Exp, bias=gb)
            prs = pmisc.tile([1, R], f32, tag="m")
            nc.tensor.matmul(prs, onescol[0:E, 0:1], expg, start=True, stop=True)
            rinv = sp.tile([1, R], f32, tag="rinv")
            nc.vector.reciprocal(rinv, prs)
            nc.vector.tensor_mul(dar, dar, rinv)

            # --- experts: accumulate over l-chunks in PSUM (bf16 inputs) ---
            eo_sb = []
            plans = [
                ([(c10, xnb), (c20, x2b)], cst[:, 0:1]),
                ([(c11, xnb), (c21, x2b)], cst[:, 1:2]),
                ([(ww0, psib)], cst[:, 2:3]),
                ([(ww1, psib)], cst[:, 3:4]),
            ]
            for ei, (mms, bcol) in enumerate(plans):
                pe_ = pacc.tile([P, R], f32, tag="acc")
                nmm = len(mms) * NCH
                i = 0
                for w, rhs in mms:
                    for c in range(NCH):
                        nc.tensor.matmul(
                            pe_,
                            w[:, c * P : (c + 1) * P],
                            rhs[:, c * R : (c + 1) * R],
                            start=(i == 0),
                            stop=(i == nmm - 1),
                        )
                        i += 1
                eo = sp.tile([P, R], f32, tag=f"eo{ei}")
                nc.vector.tensor_scalar_add(eo, pe_, bcol)
                eo_sb.append(eo)

            # --- transpose to row-major, mix experts, denorm, transpose back ---
            outp = bp.tile([P, R], f32)
            for j in range(R // PD):
                js = slice(j * PD, (j + 1) * PD)
                ptg = ptr.tile([PD, E], f32, tag="tr")
                nc.tensor.transpose(ptg, expg[:, js], ident[0:E, 0:E])
                sg = sp.tile([PD, E], f32, tag="sg")
                nc.scalar.copy(sg, ptg)
                ptd = ptr.tile([PD, 1], f32, tag="tr")
                nc.tensor.transpose(ptd, dar[:, js], ident[0:1, 0:1])
                ptd2 = ptr.tile([PD, 1], f32, tag="tr")
                nc.tensor.transpose(ptd2, dcr[:, js], ident[0:1, 0:1])
                dcol = sp.tile([PD, 2], f32, tag="dcol")
                nc.scalar.copy(dcol[:, 0:1], ptd)
                nc.scalar.copy(dcol[:, 1:2], ptd2)
                sgp = sp.tile([PD, E], f32, tag="sgp")
                nc.vector.tensor_scalar_mul(sgp, sg, dcol[:, 0:1])
                pred = sp.tile([PD, P], f32, tag="pred")
                for e in range(E):
                    pteo = ptr.tile([PD, P], f32, tag="tr")
                    nc.tensor.transpose(pteo, eo_sb[e][:, js], ident[0:P, 0:P])
                    if e == 0:
                        nc.vector.tensor_scalar(
                            pred,
                            pteo,
                            sgp[:, 0:1],
                            dcol[:, 1:2],
                            op0=OP.mult,
                            op1=OP.add,
                        )
                    else:
                        nc.vector.scalar_tensor_tensor(
                            pred, pteo, sgp[:, e : e + 1], pred, op0=OP.mult, op1=OP.add
                        )
                pto = ptr.tile([P, PD], f32, tag="tr")
                nc.tensor.transpose(pto, pred, ident)
                nc.scalar.copy(outp[:, js], pto)

            dma(out=out_d[:], in_=outp)

    nc.compile()
    return nc


def _chunked(wT):
    """[L, M] -> [128, NCH*M] with column block c holding rows l=c*128.."""
    Lx, M = wT.shape
    return np.ascontiguousarray(
        wT.reshape(NCH, PD, M).transpose(1, 0, 2).reshape(PD, NCH * M)
    )


def _host_prep(inputs):
    import ml_dtypes

    f = np.float32
    bf = ml_dtypes.bfloat16
    g = {k: np.asarray(v, f) for k, v in inputs.items()}

    bn_scale = MH / math.sqrt(1.0 + BN_EPS)
    wparts = [
        _chunked(np.ascontiguousarray(g["t0_coeffs"][:, :, 1].T)),
        _chunked(np.ascontiguousarray(g["t0_coeffs"][:, :, 2].T)),
        _chunked(np.ascontiguousarray(g["t1_coeffs"][:, :, 1].T)),
        _chunked(np.ascontiguousarray(g["t1_coeffs"][:, :, 2].T)),
        _chunked(
            np.ascontiguousarray((g["w0_ww"] * g["w0_gamma"][:, None] * bn_scale).T)
        ),
        _chunked(
            np.ascontiguousarray((g["w1_ww"] * g["w1_gamma"][:, None] * bn_scale).T)
        ),
        _chunked(np.ascontiguousarray(g["gate_w"].T)),
    ]
    onescol_h = np.ones((PD, 1), f)
    invL_h = np.full((PD, 1), 1.0 / L, f)
    onesrow_h = np.zeros((PD, PD), f)
    onesrow_h[0, :] = 1.0
    w_h = np.concatenate(wparts + [onescol_h, invL_h, onesrow_h], axis=1).astype(f)
    assert w_h.shape == (PD, WTOT)

    id_h = np.zeros((PD, PD + 2), f)
    id_h[:, 0:PD] = np.eye(PD, dtype=f)
    id_h[:, PD] = 1.0

    cst_h = np.zeros((P, 5), f)
    cst_h[:, 0] = (
        g["t0_coeffs"][:, :, 0].sum(axis=1, dtype=np.float64) + g["t0_bias"][0]
    ).astype(f)
    cst_h[:, 1] = (
        g["t1_coeffs"][:, :, 0].sum(axis=1, dtype=np.float64) + g["t1_bias"][0]
    ).astype(f)
    cst_h[:, 2] = g["w0_beta"]
    cst_h[:, 3] = g["w1_beta"]
    cst_h[0:E, 4] = g["gate_b"]

    vec_h = np.zeros((1, VECN), f)
    vec_h[0, 0:R] = np.tile(g["rev_w"], BPC)
    vec_h[0, R : 2 * R] = np.tile(g["rev_b"], BPC)
    vec_h[0, 2 * R : 3 * R] = np.tile(1.0 / (g["rev_w"] + f(EPS)), BPC)
    vec_h[0, 3 * R] = EPS
    vec_h[0, 3 * R + 1 :] = 1.0

    common = {"w": w_h, "ident": id_h, "cst": cst_h, "vec": vec_h}

    x = g["x"]
    xcores = []
    for i in range(NCORES):
        xc = x[i * BPC : (i + 1) * BPC]  # [BPC, L, N]
        xcores.append(
            np.ascontiguousarray(
                xc.reshape(BPC, NCH, PD, N).transpose(2, 1, 0, 3).reshape(PD, NCH * R)
            )
        )
    return common, xcores


def _fast_ok(inputs):
    try:
        return (
            np.all(np.asarray(inputs["w0_scale"]) == 1.0)
            and np.all(np.asarray(inputs["w1_scale"]) == 1.0)
            and np.all(np.asarray(inputs["w0_trans"]) == 0.0)
            and np.all(np.asarray(inputs["w1_trans"]) == 0.0)
        )
    except Exception:
        return False


def _numpy_ref(inputs):
    """Exact general fallback (host numpy), mirrors the reference module."""
    g = {k: np.asarray(v, np.float32) for k, v in inputs.items()}
    x = g["x"]
    mean = x.mean(axis=1, keepdims=True)
    stdev = np.sqrt(x.var(axis=1, keepdims=True) + np.float32(EPS))
    xn = (x - mean) / stdev * g["rev_w"] + g["rev_b"]
    xf = xn.transpose(0, 2, 1).reshape(B * N, L)
    logits = xf @ g["gate_w"].T + g["gate_b"]
    logits -= logits.max(axis=-1, keepdims=True)
    elg = np.exp(logits)
    score = elg / elg.sum(axis=-1, keepdims=True)

    def taylor(c, b):
        y = np.full((B * N, P), c[:, :, 0].sum(axis=1), np.float32)
        y += xf @ c[:, :, 1].T + (xf * xf) @ c[:, :, 2].T
        return y + b

    def wave(s, t, w, gam, bet):
        y = np.empty((B * N, P), np.float32)
        for i0 in range(0, B * N, 128):
            xs = (xf[i0 : i0 + 128, None, :] - t[None]) / s[None]
            x2 = xs * xs
            psi = np.float32(MH) * (x2 - 1.0) * np.exp(-0.5 * x2)
            y[i0 : i0 + 128] = np.einsum("bpl,pl->bp", psi, w)
        return (y / np.sqrt(np.float32(1.0 + BN_EPS))) * gam + bet

    eo = np.stack(
        [
            taylor(g["t0_coeffs"], g["t0_bias"][0]),
            taylor(g["t1_coeffs"], g["t1_bias"][0]),
            wave(g["w0_scale"], g["w0_trans"], g["w0_ww"], g["w0_gamma"], g["w0_beta"]),
            wave(g["w1_scale"], g["w1_trans"], g["w1_ww"], g["w1_gamma"], g["w1_beta"]),
        ],
        axis=-1,
    )
    pred = np.einsum("bpE,bE->bp", eo, score)
    pred = pred.reshape(B, N, P).transpose(0, 2, 1)
    out = ((pred - g["rev_b"]) / (g["rev_w"] + np.float32(EPS))) * stdev + mean
    return out.astype(np.float32)


def run(inputs, trace=False):
    """Run the Bass kernel on 8 cores. Returns (out [B,P,N], exec_time_ns|None)."""
    from concourse.bass_utils import run_bass_kernel_spmd

    if "nc" not in _NC_CACHE:
        _NC_CACHE["nc"] = _build_nc()
    nc = _NC_CACHE["nc"]
    common, xcores = _host_prep(inputs)
    in_maps = [dict(common, x=xcores[i]) for i in range(NCORES)]
    try:
        res = run_bass_kernel_spmd(nc, in_maps, list(range(NCORES)), trace=trace)
    except ModuleNotFoundError:
        # NTFF profiling hooks unavailable in this image — run without trace.
        res = run_bass_kernel_spmd(nc, in_maps, list(range(NCORES)), trace=False)
    out = np.empty((B, P, N), np.float32)
    for i in range(NCORES):
        o = np.asarray(res.results[i]["out"]).reshape(P, BPC, N)
        out[i * BPC : (i + 1) * BPC] = o.transpose(1, 0, 2)
    return out, res.exec_time_ns


def kernel(**inputs):
    if not _fast_ok(inputs):
        return _numpy_ref(inputs)
    out, _ = run(inputs)
    return out
